# revision 16
# baseline (speedup 1.0000x reference)
"""Trainium2 Bass kernel for nn_Cat_Linear_Encoder (pairwise MLP edge decoder).

probs[i,j] = sigmoid(W2 @ relu(W1 @ cat(z_i, z_j) + b1) + b2) * (1 - eye)

Host-side factorization (all O(N*H), exact):
    A[i,h] = |W2_h| * (z_i @ Wa.T + b1)[h]      (Wa = W1[:, :D])
    B[j,h] = |W2_h| * (z_j @ Wb.T)[h]           (Wb = W1[:, D:])
    s_h    = sign(W2_h)
    adj[i,j] = sum_h s_h * relu(A[i,h] + B[j,h]) + b2
using w*relu(x) == sign(w)*relu(|w|*x).

Device (per core, i-shard of 256 rows = 128 i-pairs):
    - R tile [128, 2048]: partitions = (pair-parity x 64 h), free = j.
      Produced by DVE tensor_scalar (fused add+relu, bf16 4x), with a
      share offloaded to ACT (activation Relu with per-partition bias)
      and GpSimd to balance engine time.
    - PE reduces h (partition axis) with a sliding 2-column sparse weight
      window, 4-way column-group tiling (tile_position): 4 concurrent
      M=32 matmuls accumulate 4 different i-pairs into one PSUM bank.
    - ACT applies sigmoid PSUM->SBUF, DMA to DRAM.
Diagonal zeroing + shard concat happen on host.
"""

import numpy as np

N, D, H = 2048, 64, 64
NCORES = 8
SHARD = N // NCORES          # 256 i-rows per core
NPAIR = SHARD // 2           # 128 i-pairs per core
IBLK = SHARD // 128          # 2 psum row-blocks per core
JCH = 512                    # j-chunk = one PSUM bank of fp32
NJC = N // JCH               # 4

# R-producer engine weights (approx per-op cost in us) for load balancing
ENG_COST = {"V": 0.87, "A": 2.05, "G": 2.45}

_CACHE = {}
_prepared_in_maps = None


def _schedule_producers():
    """Greedy assignment of the 64 ips of one iblock to engines so each
    engine's total production time is balanced."""
    counts = {e: 0.0 for e in ENG_COST}
    sched = []
    for _ in range(64):
        eng = min(ENG_COST, key=lambda e: counts[e] + ENG_COST[e])
        counts[eng] += ENG_COST[eng]
        sched.append(eng)
    return sched


def _build_bass(b2_val: float):
    import concourse.bacc as bacc
    import concourse.bass as bass
    import concourse.mybir as mybir
    from concourse.tile import TileContext

    bf16 = mybir.dt.bfloat16
    f32 = mybir.dt.float32

    nc = bacc.Bacc("TRN2", num_devices=NCORES)
    bdt_d = nc.dram_tensor("bdt", [128, N], bf16, kind="ExternalInput")
    ap_d = nc.dram_tensor("apairs", [128, NPAIR], f32, kind="ExternalInput")
    s_d = nc.dram_tensor("sbig", [128, 64], bf16, kind="ExternalInput")
    out_d = nc.dram_tensor("out", [SHARD, N], f32, kind="ExternalOutput")

    sched = _schedule_producers()

    with TileContext(nc) as tc:
        with (
            tc.tile_pool(name="const", bufs=1) as cpool,
            tc.tile_pool(name="r", bufs=10) as rpool,
            tc.tile_pool(name="o", bufs=4) as opool,
            tc.tile_pool(name="psum", bufs=8, space=bass.MemorySpace.PSUM) as ppool,
        ):
            bdt = cpool.tile([128, N], bf16, tag="bdt")
            apairs = cpool.tile([128, NPAIR], f32, tag="ap")
            sbig = cpool.tile([128, 64], bf16, tag="sbig")
            nc.sync.dma_start(out=bdt[:], in_=bdt_d[:])
            nc.sync.dma_start(out=apairs[:], in_=ap_d[:])
            nc.sync.dma_start(out=sbig[:], in_=s_d[:])

            for ib in range(IBLK):
                ps = [
                    ppool.tile([128, JCH], f32, tag="ps", name=f"ps_{ib}_{jc}")
                    for jc in range(NJC)
                ]
                for l in range(16):
                    rtiles = []
                    for b in range(4):
                        ip = ib * 64 + 16 * b + l
                        r = rpool.tile([128, N], bf16, tag="r", name=f"r_{ip}")
                        eng = sched[16 * b + l]
                        if eng == "V":
                            nc.vector.tensor_scalar(
                                out=r[:],
                                in0=bdt[:],
                                scalar1=apairs[:, ip : ip + 1],
                                scalar2=0.0,
                                op0=mybir.AluOpType.add,
                                op1=mybir.AluOpType.max,
                            )
                        elif eng == "A":
                            nc.scalar.activation(
                                r[:],
                                bdt[:],
                                mybir.ActivationFunctionType.Relu,
                                bias=apairs[:, ip : ip + 1],
                                scale=1.0,
                            )
                        else:
                            nc.gpsimd.tensor_scalar(
                                out=r[:],
                                in0=bdt[:],
                                scalar1=apairs[:, ip : ip + 1],
                                scalar2=0.0,
                                op0=mybir.AluOpType.add,
                                op1=mybir.AluOpType.max,
                            )
                        rtiles.append(r)
                    for jc in range(NJC):
                        for b in range(4):
                            nc.tensor.matmul(
                                ps[jc][32 * b : 32 * b + 32, :],
                                sbig[:, 32 - 2 * l : 64 - 2 * l],
                                rtiles[b][:, jc * JCH : (jc + 1) * JCH],
                                start=(l == 0),
                                stop=(l == 15),
                                tile_position=(0, 32 * b),
                            )
                for jc in range(NJC):
                    ot = opool.tile([128, JCH], f32, tag="ot", name=f"ot_{ib}_{jc}")
                    nc.scalar.activation(
                        ot[:],
                        ps[jc][:],
                        mybir.ActivationFunctionType.Sigmoid,
                        bias=float(b2_val),
                    )
                    nc.sync.dma_start(
                        out=out_d[ib * 128 : (ib + 1) * 128, jc * JCH : (jc + 1) * JCH],
                        in_=ot[:],
                    )
    nc.compile()
    return nc


def _default_inputs():
    """Regenerate reference setup_inputs() deterministically (CPU jax)."""
    import jax

    cpu = jax.devices("cpu")[0]
    with jax.default_device(cpu):
        key = jax.random.key(0)
        k0, k1, k2 = jax.random.split(key, 3)
        z = np.asarray(jax.random.normal(k0, (N, D), dtype="float32"))
        W1 = np.asarray(
            jax.random.normal(k1, (H, 2 * D), dtype="float32")
            * np.float32(1.0 / np.sqrt(2 * D))
        )
        b1 = np.zeros((H,), dtype=np.float32)
        W2 = np.asarray(
            jax.random.normal(k2, (1, H), dtype="float32")
            * np.float32(1.0 / np.sqrt(H))
        )
        b2 = np.zeros((1,), dtype=np.float32)
    return z, W1, b1, W2, b2


def kernel(z=None, W1=None, b1=None, W2=None, b2=None, **_unused):
    from concourse import bass_utils

    if any(x is None for x in (z, W1, b1, W2, b2)):
        dz, dW1, db1, dW2, db2 = _default_inputs()
        z = dz if z is None else np.asarray(z)
        W1 = dW1 if W1 is None else np.asarray(W1)
        b1 = db1 if b1 is None else np.asarray(b1)
        W2 = dW2 if W2 is None else np.asarray(W2)
        b2 = db2 if b2 is None else np.asarray(b2)
    z = np.asarray(z, np.float32)
    W1 = np.asarray(W1, np.float32)
    b1 = np.asarray(b1, np.float32)
    W2 = np.asarray(W2, np.float32)
    b2 = np.asarray(b2, np.float32)

    Wa, Wb = W1[:, :D], W1[:, D:]
    w2 = W2[0]                                     # [H]
    s = np.where(w2 >= 0, 1.0, -1.0).astype(np.float32)
    aw = np.abs(w2)
    A = (z @ Wa.T + b1[None, :]) * aw[None, :]     # [N, H]
    B = (z @ Wb.T) * aw[None, :]                   # [N, H]

    import ml_dtypes

    bdt = np.ascontiguousarray(
        np.concatenate([B.T, B.T], axis=0).astype(ml_dtypes.bfloat16)
    )  # [128, N]

    sbig = np.zeros((128, 64), dtype=ml_dtypes.bfloat16)
    sbig[0:64, 32] = s.astype(ml_dtypes.bfloat16)
    sbig[64:128, 33] = s.astype(ml_dtypes.bfloat16)

    # per-core A-pair columns: core c owns i in [c*SHARD, (c+1)*SHARD)
    in_maps = []
    for c in range(NCORES):
        Ash = A[c * SHARD : (c + 1) * SHARD]       # [256, H]
        ap = np.empty((128, NPAIR), dtype=np.float32)
        ap[0:64, :] = Ash[0::2].T                  # even rows of shard
        ap[64:128, :] = Ash[1::2].T                # odd rows
        in_maps.append(
            {
                "bdt": bdt,
                "apairs": np.ascontiguousarray(ap),
                "sbig": sbig,
            }
        )

    global _prepared_in_maps
    _prepared_in_maps = in_maps

    key = float(b2[0])
    if key not in _CACHE:
        _CACHE[key] = _build_bass(key)
    nc = _CACHE[key]

    res = bass_utils.run_bass_kernel_spmd(nc, in_maps, core_ids=list(range(NCORES)))
    probs = np.concatenate([r["out"] for r in res.results], axis=0)
    probs[np.arange(N), np.arange(N)] = 0.0
    return probs.astype(np.float32)


if __name__ == "__main__":
    out = kernel()
    print(out.shape, out.dtype, out[:3, :3])


# revision 17
# speedup vs baseline: 8.4320x; 8.4320x over previous
"""Trainium2 Bass kernel for nn_Cat_Linear_Encoder (pairwise MLP edge decoder).

probs[i,j] = sigmoid(W2 @ relu(W1 @ cat(z_i, z_j) + b1) + b2) * (1 - eye)

Host-side factorization (all O(N*H), exact):
    A[i,h] = |W2_h| * (z_i @ Wa.T + b1)[h]      (Wa = W1[:, :D])
    B[j,h] = |W2_h| * (z_j @ Wb.T)[h]           (Wb = W1[:, D:])
    s_h    = sign(W2_h)
    adj[i,j] = sum_h s_h * relu(A[i,h] + B[j,h]) + b2
using w*relu(x) == sign(w)*relu(|w|*x).

Device (per core, i-shard of 256 rows = 128 i-pairs):
    - R tile [128, 2048]: partitions = (pair-parity x 64 h), free = j.
      Produced by DVE tensor_scalar (fused add+relu, bf16 4x), with a
      share offloaded to ACT (activation Relu with per-partition bias)
      and GpSimd to balance engine time.
    - PE reduces h (partition axis) with a sliding 2-column sparse weight
      window, 4-way column-group tiling (tile_position): 4 concurrent
      M=32 matmuls accumulate 4 different i-pairs into one PSUM bank.
    - ACT applies sigmoid PSUM->SBUF, DMA to DRAM.
Diagonal zeroing + shard concat happen on host.
"""

import numpy as np

N, D, H = 2048, 64, 64
NCORES = 8
SHARD = N // NCORES          # 256 i-rows per core
NPAIR = SHARD // 2           # 128 i-pairs per core
IBLK = SHARD // 128          # 2 psum row-blocks per core
JCH = 512                    # j-chunk = one PSUM bank of fp32
NJC = N // JCH               # 4

# R-producer engine weights (approx per-op cost in us) for load balancing.
# GpSimd tensor ops measured ~30us/op on HW (SBUF port contention with DVE)
# so it is excluded.
ENG_COST = {"V": 0.87, "A": 2.05}

_CACHE = {}
_prepared_in_maps = None


def _schedule_producers():
    """Greedy assignment of the 64 ips of one iblock to engines so each
    engine's total production time is balanced."""
    counts = {e: 0.0 for e in ENG_COST}
    sched = []
    for _ in range(64):
        eng = min(ENG_COST, key=lambda e: counts[e] + ENG_COST[e])
        counts[eng] += ENG_COST[eng]
        sched.append(eng)
    return sched


def _build_bass(b2_val: float):
    import concourse.bacc as bacc
    import concourse.bass as bass
    import concourse.mybir as mybir
    from concourse.tile import TileContext

    bf16 = mybir.dt.bfloat16
    f32 = mybir.dt.float32

    nc = bacc.Bacc("TRN2", num_devices=NCORES)
    bdt_d = nc.dram_tensor("bdt", [128, N], bf16, kind="ExternalInput")
    ap_d = nc.dram_tensor("apairs", [128, NPAIR], f32, kind="ExternalInput")
    s_d = nc.dram_tensor("sbig", [128, 64], bf16, kind="ExternalInput")
    out_d = nc.dram_tensor("out", [SHARD, N], f32, kind="ExternalOutput")

    sched = _schedule_producers()

    with TileContext(nc) as tc:
        with (
            tc.tile_pool(name="const", bufs=1) as cpool,
            tc.tile_pool(name="r", bufs=10) as rpool,
            tc.tile_pool(name="o", bufs=4) as opool,
            tc.tile_pool(name="psum", bufs=8, space=bass.MemorySpace.PSUM) as ppool,
        ):
            bdt = cpool.tile([128, N], bf16, tag="bdt")
            apairs = cpool.tile([128, NPAIR], f32, tag="ap")
            sbig = cpool.tile([128, 64], bf16, tag="sbig")
            nc.sync.dma_start(out=bdt[:], in_=bdt_d[:])
            nc.sync.dma_start(out=apairs[:], in_=ap_d[:])
            nc.sync.dma_start(out=sbig[:], in_=s_d[:])

            for ib in range(IBLK):
                ps = [
                    ppool.tile([128, JCH], f32, tag="ps", name=f"ps_{ib}_{jc}")
                    for jc in range(NJC)
                ]
                for l in range(16):
                    rtiles = []
                    for b in range(4):
                        ip = ib * 64 + 16 * b + l
                        r = rpool.tile([128, N], bf16, tag="r", name=f"r_{ip}")
                        eng = sched[16 * b + l]
                        if eng == "V":
                            nc.vector.tensor_scalar(
                                out=r[:],
                                in0=bdt[:],
                                scalar1=apairs[:, ip : ip + 1],
                                scalar2=0.0,
                                op0=mybir.AluOpType.add,
                                op1=mybir.AluOpType.max,
                            )
                        elif eng == "A":
                            nc.scalar.activation(
                                r[:],
                                bdt[:],
                                mybir.ActivationFunctionType.Relu,
                                bias=apairs[:, ip : ip + 1],
                                scale=1.0,
                            )
                        else:
                            nc.gpsimd.tensor_scalar(
                                out=r[:],
                                in0=bdt[:],
                                scalar1=apairs[:, ip : ip + 1],
                                scalar2=0.0,
                                op0=mybir.AluOpType.add,
                                op1=mybir.AluOpType.max,
                            )
                        rtiles.append(r)
                    for jc in range(NJC):
                        for b in range(4):
                            nc.tensor.matmul(
                                ps[jc][32 * b : 32 * b + 32, :],
                                sbig[:, 32 - 2 * l : 64 - 2 * l],
                                rtiles[b][:, jc * JCH : (jc + 1) * JCH],
                                start=(l == 0),
                                stop=(l == 15),
                                tile_position=(0, 32 * b),
                            )
                for jc in range(NJC):
                    ot = opool.tile([128, JCH], f32, tag="ot", name=f"ot_{ib}_{jc}")
                    nc.scalar.activation(
                        ot[:],
                        ps[jc][:],
                        mybir.ActivationFunctionType.Sigmoid,
                        bias=float(b2_val),
                    )
                    nc.sync.dma_start(
                        out=out_d[ib * 128 : (ib + 1) * 128, jc * JCH : (jc + 1) * JCH],
                        in_=ot[:],
                    )
    nc.compile()
    return nc


def _default_inputs():
    """Regenerate reference setup_inputs() deterministically (CPU jax)."""
    import jax

    cpu = jax.devices("cpu")[0]
    with jax.default_device(cpu):
        key = jax.random.key(0)
        k0, k1, k2 = jax.random.split(key, 3)
        z = np.asarray(jax.random.normal(k0, (N, D), dtype="float32"))
        W1 = np.asarray(
            jax.random.normal(k1, (H, 2 * D), dtype="float32")
            * np.float32(1.0 / np.sqrt(2 * D))
        )
        b1 = np.zeros((H,), dtype=np.float32)
        W2 = np.asarray(
            jax.random.normal(k2, (1, H), dtype="float32")
            * np.float32(1.0 / np.sqrt(H))
        )
        b2 = np.zeros((1,), dtype=np.float32)
    return z, W1, b1, W2, b2


def kernel(z=None, W1=None, b1=None, W2=None, b2=None, **_unused):
    from concourse import bass_utils

    if any(x is None for x in (z, W1, b1, W2, b2)):
        dz, dW1, db1, dW2, db2 = _default_inputs()
        z = dz if z is None else np.asarray(z)
        W1 = dW1 if W1 is None else np.asarray(W1)
        b1 = db1 if b1 is None else np.asarray(b1)
        W2 = dW2 if W2 is None else np.asarray(W2)
        b2 = db2 if b2 is None else np.asarray(b2)
    z = np.asarray(z, np.float32)
    W1 = np.asarray(W1, np.float32)
    b1 = np.asarray(b1, np.float32)
    W2 = np.asarray(W2, np.float32)
    b2 = np.asarray(b2, np.float32)

    Wa, Wb = W1[:, :D], W1[:, D:]
    w2 = W2[0]                                     # [H]
    s = np.where(w2 >= 0, 1.0, -1.0).astype(np.float32)
    aw = np.abs(w2)
    A = (z @ Wa.T + b1[None, :]) * aw[None, :]     # [N, H]
    B = (z @ Wb.T) * aw[None, :]                   # [N, H]

    import ml_dtypes

    bdt = np.ascontiguousarray(
        np.concatenate([B.T, B.T], axis=0).astype(ml_dtypes.bfloat16)
    )  # [128, N]

    sbig = np.zeros((128, 64), dtype=ml_dtypes.bfloat16)
    sbig[0:64, 32] = s.astype(ml_dtypes.bfloat16)
    sbig[64:128, 33] = s.astype(ml_dtypes.bfloat16)

    # per-core A-pair columns: core c owns i in [c*SHARD, (c+1)*SHARD)
    in_maps = []
    for c in range(NCORES):
        Ash = A[c * SHARD : (c + 1) * SHARD]       # [256, H]
        ap = np.empty((128, NPAIR), dtype=np.float32)
        ap[0:64, :] = Ash[0::2].T                  # even rows of shard
        ap[64:128, :] = Ash[1::2].T                # odd rows
        in_maps.append(
            {
                "bdt": bdt,
                "apairs": np.ascontiguousarray(ap),
                "sbig": sbig,
            }
        )

    global _prepared_in_maps
    _prepared_in_maps = in_maps

    key = float(b2[0])
    if key not in _CACHE:
        _CACHE[key] = _build_bass(key)
    nc = _CACHE[key]

    res = bass_utils.run_bass_kernel_spmd(nc, in_maps, core_ids=list(range(NCORES)))
    probs = np.concatenate([r["out"] for r in res.results], axis=0)
    probs[np.arange(N), np.arange(N)] = 0.0
    return probs.astype(np.float32)


if __name__ == "__main__":
    out = kernel()
    print(out.shape, out.dtype, out[:3, :3])


# revision 19
# speedup vs baseline: 8.8722x; 1.0522x over previous
"""Trainium2 Bass kernel for nn_Cat_Linear_Encoder (pairwise MLP edge decoder).

probs[i,j] = sigmoid(W2 @ relu(W1 @ cat(z_i, z_j) + b1) + b2) * (1 - eye)

Host-side factorization (all O(N*H), exact):
    A[i,h] = |W2_h| * (z_i @ Wa.T + b1)[h]      (Wa = W1[:, :D])
    B[j,h] = |W2_h| * (z_j @ Wb.T)[h]           (Wb = W1[:, D:])
    s_h    = sign(W2_h)
    adj[i,j] = sum_h s_h * relu(A[i,h] + B[j,h]) + b2
using w*relu(x) == sign(w)*relu(|w|*x).

Device (per core, i-shard of 256 rows = 128 i-pairs):
    - R tile [128, 2048]: partitions = (pair-parity x 64 h), free = j.
      Produced by DVE tensor_scalar (fused add+relu, bf16 4x), with a
      share offloaded to ACT (activation Relu with per-partition bias)
      and GpSimd to balance engine time.
    - PE reduces h (partition axis) with a sliding 2-column sparse weight
      window, 4-way column-group tiling (tile_position): 4 concurrent
      M=32 matmuls accumulate 4 different i-pairs into one PSUM bank.
    - ACT applies sigmoid PSUM->SBUF, DMA to DRAM.
Diagonal zeroing + shard concat happen on host.
"""

import numpy as np

N, D, H = 2048, 64, 64
NCORES = 8
SHARD = N // NCORES          # 256 i-rows per core
NPAIR = SHARD // 2           # 128 i-pairs per core
IBLK = SHARD // 128          # 2 psum row-blocks per core
JCH = 512                    # j-chunk = one PSUM bank of fp32
NJC = N // JCH               # 4

# R-producer engine weights (approx per-op cost in us) for load balancing.
# GpSimd tensor ops measured ~30us/op on HW (SBUF port contention with DVE)
# so it is excluded.
ENG_COST = {"V": 0.845, "A": 2.15}

_CACHE = {}
_prepared_in_maps = None


def _schedule_producers():
    """Greedy assignment of the 64 ips of one iblock to engines so each
    engine's total production time is balanced."""
    counts = {e: 0.0 for e in ENG_COST}
    sched = []
    for _ in range(64):
        eng = min(ENG_COST, key=lambda e: counts[e] + ENG_COST[e])
        counts[eng] += ENG_COST[eng]
        sched.append(eng)
    return sched


def _build_bass(b2_val: float):
    import concourse.bacc as bacc
    import concourse.bass as bass
    import concourse.mybir as mybir
    from concourse.tile import TileContext

    bf16 = mybir.dt.bfloat16
    f32 = mybir.dt.float32

    nc = bacc.Bacc("TRN2", num_devices=NCORES)
    bdt_d = nc.dram_tensor("bdt", [128, N], bf16, kind="ExternalInput")
    ap_d = nc.dram_tensor("apairs", [128, NPAIR], f32, kind="ExternalInput")
    s_d = nc.dram_tensor("sbig", [128, 64], bf16, kind="ExternalInput")
    out_d = nc.dram_tensor("out", [SHARD, N], f32, kind="ExternalOutput")

    sched = _schedule_producers()

    with TileContext(nc) as tc:
        with (
            tc.tile_pool(name="const", bufs=1) as cpool,
            tc.tile_pool(name="r", bufs=10) as rpool,
            tc.tile_pool(name="o", bufs=4) as opool,
            tc.tile_pool(name="psum", bufs=8, space=bass.MemorySpace.PSUM) as ppool,
        ):
            bdt = cpool.tile([128, N], bf16, tag="bdt")
            apairs = cpool.tile([128, NPAIR], f32, tag="ap")
            sbig = cpool.tile([128, 64], bf16, tag="sbig")
            nc.sync.dma_start(out=bdt[:], in_=bdt_d[:])
            nc.gpsimd.dma_start(out=apairs[:], in_=ap_d[:])
            nc.gpsimd.dma_start(out=sbig[:], in_=s_d[:])

            # dummy sigmoid on a scratch tile: front-loads the ACT table set
            # (includes relu) in parallel with the input DMAs
            warm = cpool.tile([128, 1], f32, tag="warm")
            nc.vector.memset(warm[:], 0.0)
            nc.scalar.activation(
                warm[:], warm[:], mybir.ActivationFunctionType.Sigmoid, bias=0.0
            )

            for ib in range(IBLK):
                ps = [
                    ppool.tile([128, JCH], f32, tag="ps", name=f"ps_{ib}_{jc}")
                    for jc in range(NJC)
                ]
                for l in range(16):
                    rtiles = []
                    for b in range(4):
                        ip = ib * 64 + 16 * b + l
                        r = rpool.tile([128, N], bf16, tag="r", name=f"r_{ip}")
                        eng = sched[16 * b + l]
                        if eng == "V":
                            nc.vector.tensor_scalar(
                                out=r[:],
                                in0=bdt[:],
                                scalar1=apairs[:, ip : ip + 1],
                                scalar2=0.0,
                                op0=mybir.AluOpType.add,
                                op1=mybir.AluOpType.max,
                            )
                        elif eng == "A":
                            nc.scalar.activation(
                                r[:],
                                bdt[:],
                                mybir.ActivationFunctionType.Relu,
                                bias=apairs[:, ip : ip + 1],
                                scale=1.0,
                            )
                        else:
                            nc.gpsimd.tensor_scalar(
                                out=r[:],
                                in0=bdt[:],
                                scalar1=apairs[:, ip : ip + 1],
                                scalar2=0.0,
                                op0=mybir.AluOpType.add,
                                op1=mybir.AluOpType.max,
                            )
                        rtiles.append(r)
                    for jc in range(NJC):
                        for b in range(4):
                            nc.tensor.matmul(
                                ps[jc][32 * b : 32 * b + 32, :],
                                sbig[:, 32 - 2 * l : 64 - 2 * l],
                                rtiles[b][:, jc * JCH : (jc + 1) * JCH],
                                start=(l == 0),
                                stop=(l == 15),
                                tile_position=(0, 32 * b),
                            )
                for jc in range(NJC):
                    ot = opool.tile([128, JCH], f32, tag="ot", name=f"ot_{ib}_{jc}")
                    nc.scalar.activation(
                        ot[:],
                        ps[jc][:],
                        mybir.ActivationFunctionType.Sigmoid,
                        bias=float(b2_val),
                    )
                    nc.sync.dma_start(
                        out=out_d[ib * 128 : (ib + 1) * 128, jc * JCH : (jc + 1) * JCH],
                        in_=ot[:],
                    )
    nc.compile()
    return nc


def _default_inputs():
    """Regenerate reference setup_inputs() deterministically (CPU jax)."""
    import jax

    cpu = jax.devices("cpu")[0]
    with jax.default_device(cpu):
        key = jax.random.key(0)
        k0, k1, k2 = jax.random.split(key, 3)
        z = np.asarray(jax.random.normal(k0, (N, D), dtype="float32"))
        W1 = np.asarray(
            jax.random.normal(k1, (H, 2 * D), dtype="float32")
            * np.float32(1.0 / np.sqrt(2 * D))
        )
        b1 = np.zeros((H,), dtype=np.float32)
        W2 = np.asarray(
            jax.random.normal(k2, (1, H), dtype="float32")
            * np.float32(1.0 / np.sqrt(H))
        )
        b2 = np.zeros((1,), dtype=np.float32)
    return z, W1, b1, W2, b2


def kernel(z=None, W1=None, b1=None, W2=None, b2=None, **_unused):
    from concourse import bass_utils

    if any(x is None for x in (z, W1, b1, W2, b2)):
        dz, dW1, db1, dW2, db2 = _default_inputs()
        z = dz if z is None else np.asarray(z)
        W1 = dW1 if W1 is None else np.asarray(W1)
        b1 = db1 if b1 is None else np.asarray(b1)
        W2 = dW2 if W2 is None else np.asarray(W2)
        b2 = db2 if b2 is None else np.asarray(b2)
    z = np.asarray(z, np.float32)
    W1 = np.asarray(W1, np.float32)
    b1 = np.asarray(b1, np.float32)
    W2 = np.asarray(W2, np.float32)
    b2 = np.asarray(b2, np.float32)

    Wa, Wb = W1[:, :D], W1[:, D:]
    w2 = W2[0]                                     # [H]
    s = np.where(w2 >= 0, 1.0, -1.0).astype(np.float32)
    aw = np.abs(w2)
    A = (z @ Wa.T + b1[None, :]) * aw[None, :]     # [N, H]
    B = (z @ Wb.T) * aw[None, :]                   # [N, H]

    import ml_dtypes

    bdt = np.ascontiguousarray(
        np.concatenate([B.T, B.T], axis=0).astype(ml_dtypes.bfloat16)
    )  # [128, N]

    sbig = np.zeros((128, 64), dtype=ml_dtypes.bfloat16)
    sbig[0:64, 32] = s.astype(ml_dtypes.bfloat16)
    sbig[64:128, 33] = s.astype(ml_dtypes.bfloat16)

    # per-core A-pair columns: core c owns i in [c*SHARD, (c+1)*SHARD)
    in_maps = []
    for c in range(NCORES):
        Ash = A[c * SHARD : (c + 1) * SHARD]       # [256, H]
        ap = np.empty((128, NPAIR), dtype=np.float32)
        ap[0:64, :] = Ash[0::2].T                  # even rows of shard
        ap[64:128, :] = Ash[1::2].T                # odd rows
        in_maps.append(
            {
                "bdt": bdt,
                "apairs": np.ascontiguousarray(ap),
                "sbig": sbig,
            }
        )

    global _prepared_in_maps
    _prepared_in_maps = in_maps

    key = float(b2[0])
    if key not in _CACHE:
        _CACHE[key] = _build_bass(key)
    nc = _CACHE[key]

    res = bass_utils.run_bass_kernel_spmd(nc, in_maps, core_ids=list(range(NCORES)))
    probs = np.concatenate([r["out"] for r in res.results], axis=0)
    probs[np.arange(N), np.arange(N)] = 0.0
    return probs.astype(np.float32)


if __name__ == "__main__":
    out = kernel()
    print(out.shape, out.dtype, out[:3, :3])


# revision 24
# speedup vs baseline: 9.3576x; 1.0547x over previous
"""Trainium2 Bass kernel for nn_Cat_Linear_Encoder (pairwise MLP edge decoder).

probs[i,j] = sigmoid(W2 @ relu(W1 @ cat(z_i, z_j) + b1) + b2) * (1 - eye)

Host-side factorization (all O(N*H), exact):
    A[i,h] = |W2_h| * (z_i @ Wa.T + b1)[h]      (Wa = W1[:, :D])
    B[j,h] = |W2_h| * (z_j @ Wb.T)[h]           (Wb = W1[:, D:])
    s_h    = sign(W2_h)
    adj[i,j] = sum_h s_h * relu(A[i,h] + B[j,h]) + b2
using w*relu(x) == sign(w)*relu(|w|*x).

Device (per core, i-shard of 256 rows = 128 i-pairs):
    - R tile [128, 2048]: partitions = (pair-parity x 64 h), free = j.
      Produced by DVE tensor_scalar (fused add+relu, bf16 4x), with a
      share offloaded to ACT (activation Relu with per-partition bias)
      and GpSimd to balance engine time.
    - PE reduces h (partition axis) with a sliding 2-column sparse weight
      window, 4-way column-group tiling (tile_position): 4 concurrent
      M=32 matmuls accumulate 4 different i-pairs into one PSUM bank.
    - ACT applies sigmoid PSUM->SBUF, DMA to DRAM.
Diagonal zeroing + shard concat happen on host.
"""

import numpy as np

N, D, H = 2048, 64, 64
NCORES = 8
SHARD = N // NCORES          # 256 i-rows per core
NPAIR = SHARD // 2           # 128 i-pairs per core
IBLK = SHARD // 128          # 2 psum row-blocks per core
JCH = 512                    # j-chunk = one PSUM bank of fp32
NJC = N // JCH               # 4

# R-producer engine weights (approx per-op cost in us) for load balancing.
# GpSimd tensor ops measured ~30us/op on HW (SBUF port contention with DVE)
# so it is excluded.
ENG_COST = {"V": 0.748, "A": 2.0}
# ACT also runs the sigmoid epilogue + table load (~3.5us per iblock), so its
# producer budget starts pre-charged.
ENG_INIT = {"V": 0.0, "A": 3.5}

_CACHE = {}
_prepared_in_maps = None


def _schedule_producers():
    """Greedy assignment of the 64 ips of one iblock to engines so each
    engine's total production time is balanced."""
    counts = dict(ENG_INIT)
    sched = []
    for _ in range(64):
        eng = min(ENG_COST, key=lambda e: counts[e] + ENG_COST[e])
        counts[eng] += ENG_COST[eng]
        sched.append(eng)
    return sched


def _build_bass(b2_val: float):
    import concourse.bacc as bacc
    import concourse.bass as bass
    import concourse.mybir as mybir
    from concourse.tile import TileContext

    bf16 = mybir.dt.bfloat16
    f32 = mybir.dt.float32

    nc = bacc.Bacc("TRN2", num_devices=NCORES)
    bdt_d = nc.dram_tensor("bdt", [64, N], bf16, kind="ExternalInput")
    ap_d = nc.dram_tensor("apairs", [128, NPAIR], f32, kind="ExternalInput")
    s_d = nc.dram_tensor("sbig", [128, 64], bf16, kind="ExternalInput")
    out_d = nc.dram_tensor("out", [SHARD, N], f32, kind="ExternalOutput")

    sched = _schedule_producers()

    with TileContext(nc) as tc:
        with (
            tc.tile_pool(name="const", bufs=1) as cpool,
            tc.tile_pool(name="r", bufs=10) as rpool,
            tc.tile_pool(name="o", bufs=4) as opool,
            tc.tile_pool(name="psum", bufs=8, space=bass.MemorySpace.PSUM) as ppool,
        ):
            bdt = cpool.tile([128, N], bf16, tag="bdt")
            apairs = cpool.tile([128, NPAIR], f32, tag="ap")
            sbig = cpool.tile([128, 64], bf16, tag="sbig")
            nc.sync.dma_start(out=bdt[0:64, :], in_=bdt_d[:])
            nc.gpsimd.dma_start(out=bdt[64:128, :], in_=bdt_d[:])
            nc.sync.dma_start(out=apairs[:], in_=ap_d[:])
            nc.gpsimd.dma_start(out=sbig[:], in_=s_d[:])

            # dummy sigmoid on a scratch tile: front-loads the ACT table set
            # (includes relu) in parallel with the input DMAs
            warm = cpool.tile([128, 1], f32, tag="warm")
            nc.vector.memset(warm[:], 0.0)
            nc.scalar.activation(
                warm[:], warm[:], mybir.ActivationFunctionType.Sigmoid, bias=0.0
            )

            for ib in range(IBLK):
                ps = [
                    ppool.tile([128, JCH], f32, tag="ps", name=f"ps_{ib}_{jc}")
                    for jc in range(NJC)
                ]
                for l in range(16):
                    rtiles = []
                    for b in range(4):
                        ip = ib * 64 + 16 * b + l
                        r = rpool.tile([128, N], bf16, tag="r", name=f"r_{ip}")
                        eng = sched[16 * b + l]
                        if eng == "V":
                            nc.vector.tensor_scalar(
                                out=r[:],
                                in0=bdt[:],
                                scalar1=apairs[:, ip : ip + 1],
                                scalar2=0.0,
                                op0=mybir.AluOpType.add,
                                op1=mybir.AluOpType.max,
                            )
                        elif eng == "A":
                            nc.scalar.activation(
                                r[:],
                                bdt[:],
                                mybir.ActivationFunctionType.Relu,
                                bias=apairs[:, ip : ip + 1],
                                scale=1.0,
                            )
                        else:
                            nc.gpsimd.tensor_scalar(
                                out=r[:],
                                in0=bdt[:],
                                scalar1=apairs[:, ip : ip + 1],
                                scalar2=0.0,
                                op0=mybir.AluOpType.add,
                                op1=mybir.AluOpType.max,
                            )
                        rtiles.append(r)
                    for jc in range(NJC):
                        for b in range(4):
                            nc.tensor.matmul(
                                ps[jc][32 * b : 32 * b + 32, :],
                                sbig[:, 32 - 2 * l : 64 - 2 * l],
                                rtiles[b][:, jc * JCH : (jc + 1) * JCH],
                                start=(l == 0),
                                stop=(l == 15),
                                tile_position=(0, 32 * b),
                            )
                for jc in range(NJC):
                    ot = opool.tile([128, JCH], f32, tag="ot", name=f"ot_{ib}_{jc}")
                    nc.scalar.activation(
                        ot[:],
                        ps[jc][:],
                        mybir.ActivationFunctionType.Sigmoid,
                        bias=float(b2_val),
                    )
                    nc.sync.dma_start(
                        out=out_d[ib * 128 : (ib + 1) * 128, jc * JCH : (jc + 1) * JCH],
                        in_=ot[:],
                    )
    nc.compile()
    return nc


def _default_inputs():
    """Regenerate reference setup_inputs() deterministically (CPU jax)."""
    import jax

    cpu = jax.devices("cpu")[0]
    with jax.default_device(cpu):
        key = jax.random.key(0)
        k0, k1, k2 = jax.random.split(key, 3)
        z = np.asarray(jax.random.normal(k0, (N, D), dtype="float32"))
        W1 = np.asarray(
            jax.random.normal(k1, (H, 2 * D), dtype="float32")
            * np.float32(1.0 / np.sqrt(2 * D))
        )
        b1 = np.zeros((H,), dtype=np.float32)
        W2 = np.asarray(
            jax.random.normal(k2, (1, H), dtype="float32")
            * np.float32(1.0 / np.sqrt(H))
        )
        b2 = np.zeros((1,), dtype=np.float32)
    return z, W1, b1, W2, b2


def kernel(z=None, W1=None, b1=None, W2=None, b2=None, **_unused):
    from concourse import bass_utils

    if any(x is None for x in (z, W1, b1, W2, b2)):
        dz, dW1, db1, dW2, db2 = _default_inputs()
        z = dz if z is None else np.asarray(z)
        W1 = dW1 if W1 is None else np.asarray(W1)
        b1 = db1 if b1 is None else np.asarray(b1)
        W2 = dW2 if W2 is None else np.asarray(W2)
        b2 = db2 if b2 is None else np.asarray(b2)
    z = np.asarray(z, np.float32)
    W1 = np.asarray(W1, np.float32)
    b1 = np.asarray(b1, np.float32)
    W2 = np.asarray(W2, np.float32)
    b2 = np.asarray(b2, np.float32)

    Wa, Wb = W1[:, :D], W1[:, D:]
    w2 = W2[0]                                     # [H]
    s = np.where(w2 >= 0, 1.0, -1.0).astype(np.float32)
    aw = np.abs(w2)
    A = (z @ Wa.T + b1[None, :]) * aw[None, :]     # [N, H]
    B = (z @ Wb.T) * aw[None, :]                   # [N, H]

    import ml_dtypes

    bdt = np.ascontiguousarray(B.T.astype(ml_dtypes.bfloat16))  # [64, N]

    sbig = np.zeros((128, 64), dtype=ml_dtypes.bfloat16)
    sbig[0:64, 32] = s.astype(ml_dtypes.bfloat16)
    sbig[64:128, 33] = s.astype(ml_dtypes.bfloat16)

    # per-core A-pair columns: core c owns i in [c*SHARD, (c+1)*SHARD)
    in_maps = []
    for c in range(NCORES):
        Ash = A[c * SHARD : (c + 1) * SHARD]       # [256, H]
        ap = np.empty((128, NPAIR), dtype=np.float32)
        ap[0:64, :] = Ash[0::2].T                  # even rows of shard
        ap[64:128, :] = Ash[1::2].T                # odd rows
        in_maps.append(
            {
                "bdt": bdt,
                "apairs": np.ascontiguousarray(ap),
                "sbig": sbig,
            }
        )

    global _prepared_in_maps
    _prepared_in_maps = in_maps

    key = float(b2[0])
    if key not in _CACHE:
        _CACHE[key] = _build_bass(key)
    nc = _CACHE[key]

    res = bass_utils.run_bass_kernel_spmd(nc, in_maps, core_ids=list(range(NCORES)))
    probs = np.concatenate([r["out"] for r in res.results], axis=0)
    probs[np.arange(N), np.arange(N)] = 0.0
    return probs.astype(np.float32)


if __name__ == "__main__":
    out = kernel()
    print(out.shape, out.dtype, out[:3, :3])
